# revision 13
# baseline (speedup 1.0000x reference)
"""Trainium2 Bass kernel for nn_Eq1dConv (conv1d(K=3)+bias -> filtered_lrelu).

Math (all separable along W; H untouched because the 2x up/down in H uses a
1-tap filter so inserted zero rows are dropped again by the ::2 decimate):

  y_b[co,h,m]  = sum_{ci,k} x[ci,h,m+k-1]*w[co,ci,k] + b[co]      (m in [0,512))
  pre_a[m'] = fk1*(y_b[m'-1]+y_b[m'])                  (up-FIR even phase, fk1==fk3)
  pre_b[m'] = fk0*(y_b[m'-1]+y_b[m'+1]) + fk2*y_b[m']  (odd phase, fk0==fk4)
  out[n] = fd0*lr(pre_a[n]) + fd1*lr(pre_b[n]) + fd2*lr(pre_a[n+1]) + fd3*lr(pre_b[n+1])

with lr = leaky-relu(0.2), fk = 4*flip(up_filter), fd = flip(down_filter).
lr(c*u) = c*max(u,0.2u) for c>0 (min for c<0), so the fk/fd scales fold into
the diagonal matmuls of the final comb.

Per row-pair of h-rows (h, h+32), partitions q = 2*ci+g / 2*co+g:
  PE   : 7 fp16 matmuls -> y_b PSUM [128, 515] (2 banks, bias+boundary-zeroing
         via a Kc=1 pattern matmul), later 4 diagonal fp16 matmuls for the comb
  ACT  : evict y_b -> fp16 SBUF
  DVE  : s_a, s_b0 shift-adds; u = ratio*y16 + s_b0 (fused STT); B2 lrelu-core
  GPSIMD: A2 lrelu-core (SBUF-only engine)

Sharding: pure data-parallel, batch 8 -> 8 cores, weights replicated.
"""

import numpy as np
from contextlib import ExitStack

import concourse.bass as bass
import concourse.bacc as bacc
import concourse.mybir as mybir
import concourse.tile as tile
from concourse.bass_utils import run_bass_kernel_spmd

B, CIN, COUT, H, W, K = 8, 64, 64, 64, 512, 3
N_CORES = 8
SLOPE = 0.2

F32 = mybir.dt.float32
F16 = mybir.dt.float16


def _alu(c):
    # lr(c*u) = c * (max if c > 0 else min)(u, SLOPE*u)
    return mybir.AluOpType.max if c > 0 else mybir.AluOpType.min


def build_program(n_rowpairs=H // 2, rp_per_gran=4):
    """Build the single-core SPMD program. Returns (nc, go)."""
    nc = bacc.Bacc("TRN2", target_bir_lowering=False, debug=False)

    x_d = nc.declare_dram_parameter("x", [CIN, H, W], F32, isOutput=False)
    wb_d = nc.declare_dram_parameter("wb", [K, 128, 128], F16, isOutput=False)
    brow_d = nc.declare_dram_parameter("brow", [1, 128], F16, isOutput=False)
    pat_d = nc.declare_dram_parameter("pat", [1, 515], F16, isOutput=False)
    dg_d = nc.declare_dram_parameter("dg", [4, 128, 128], F16, isOutput=False)
    out_d = nc.declare_dram_parameter("out", [COUT, H, W], F32, isOutput=True)

    n_gran = (n_rowpairs + rp_per_gran - 1) // rp_per_gran

    def go(ratio, alu_a, alu_b):
        with tile.TileContext(nc) as tc, ExitStack() as ctx:
            cpool = ctx.enter_context(tc.tile_pool(name="consts", bufs=1))
            xpool = ctx.enter_context(tc.tile_pool(name="xg", bufs=2))
            opool = ctx.enter_context(tc.tile_pool(name="og", bufs=2))
            wpool = ctx.enter_context(tc.tile_pool(name="work", bufs=3))
            ypool = ctx.enter_context(
                tc.tile_pool(name="ypsum", bufs=2, space=bass.MemorySpace.PSUM)
            )
            fpool = ctx.enter_context(
                tc.tile_pool(name="fpsum", bufs=2, space=bass.MemorySpace.PSUM)
            )

            wb_t = []
            for k in range(K):
                t = cpool.tile([128, 128], F16, tag=f"wb{k}")
                nc.sync.dma_start(t[:], wb_d[k])
                wb_t.append(t)
            dg_t = []
            for k in range(4):
                t = cpool.tile([128, 128], F16, tag=f"dg{k}")
                nc.sync.dma_start(t[:], dg_d[k])
                dg_t.append(t)
            brow = cpool.tile([1, 128], F16, tag="brow")
            nc.sync.dma_start(brow[:], brow_d[:])
            pat = cpool.tile([1, 515], F16, tag="pat")
            nc.sync.dma_start(pat[:], pat_d[:])

            mm = lambda o_, l_, r_, s1, s2: nc.tensor.matmul(
                o_, l_, r_, start=s1, stop=s2
            )

            x_view = x_d.rearrange("c (p hh) w -> (c p) hh w", p=2)
            o_view = out_d.rearrange("c (p hh) w -> (c p) hh w", p=2)

            for g in range(n_gran):
                rp0 = g * rp_per_gran
                nrp = min(rp_per_gran, n_rowpairs - rp0)
                xg = xpool.tile([128, rp_per_gran, W], F16, tag="xg")
                # SWDGE dma casts f32 -> f16 in flight
                nc.gpsimd.dma_start(xg[:, 0:nrp, :], x_view[:, rp0 : rp0 + nrp, :])
                og = opool.tile([128, rp_per_gran, W], F32, tag="og")

                for j in range(nrp):
                    xs = xg[:, j, :]
                    # y_b in PSUM: col jj <-> m = jj-1, window m in [-1, 513]
                    y = ypool.tile([128, 1024], F32, tag="y")
                    # bias (+ zeroing of invalid m) first, widest range per bank
                    mm(y[:, 0:512], brow[:], pat[:, 0:512], True, False)
                    mm(y[:, 512:515], brow[:], pat[:, 512:515], True, False)
                    # tap k=1: y[jj] += w1 @ x[jj-1]
                    mm(y[:, 1:512], wb_t[1][:], xs[:, 0:511], False, False)
                    mm(y[:, 512:513], wb_t[1][:], xs[:, 511:512], False, False)
                    # tap k=2: y[jj] += w2 @ x[jj]   (m <= 510)
                    mm(y[:, 1:512], wb_t[2][:], xs[:, 1:512], False, False)
                    # tap k=0: y[jj] += w0 @ x[jj-2]
                    mm(y[:, 2:512], wb_t[0][:], xs[:, 0:510], False, True)
                    mm(y[:, 512:513], wb_t[0][:], xs[:, 510:511], False, True)

                    # evict y -> fp16 SBUF (ACT), makes the DVE stages 2x
                    y16 = wpool.tile([128, 515], F16, tag="y16")
                    nc.scalar.copy(y16[:], y[:, 0:515])

                    # s_a[m'] = y16[m'-1] + y16[m'],  m' in [0, 512]  (GPSIMD)
                    s_a = wpool.tile([128, 513], F16, tag="s_a")
                    nc.gpsimd.tensor_tensor(
                        s_a[:], y16[:, 0:513], y16[:, 1:514], mybir.AluOpType.add
                    )
                    # s_b0[m'] = y16[m'-1] + y16[m'+1]
                    s_b0 = wpool.tile([128, 513], F16, tag="s_b0")
                    nc.vector.tensor_tensor(
                        s_b0[:], y16[:, 0:513], y16[:, 2:515], mybir.AluOpType.add
                    )
                    # u = ratio*y16[m'] + s_b0    (= pre_b / fk0)
                    u = wpool.tile([128, 513], F16, tag="u")
                    nc.vector.scalar_tensor_tensor(
                        u[:],
                        y16[:, 1:514],
                        float(ratio),
                        s_b0[:],
                        mybir.AluOpType.mult,
                        mybir.AluOpType.add,
                    )
                    # A2 = (max|min)(s_a, SLOPE*s_a)  -> lr(pre_a) = fk1*A2
                    a2 = wpool.tile([128, 513], F16, tag="a2")
                    nc.vector.scalar_tensor_tensor(
                        a2[:], s_a[:], SLOPE, s_a[:], mybir.AluOpType.mult, alu_a
                    )
                    # B2 = (max|min)(u, SLOPE*u)      -> lr(pre_b) = fk0*B2
                    b2 = wpool.tile([128, 513], F16, tag="b2")
                    nc.vector.scalar_tensor_tensor(
                        b2[:], u[:], SLOPE, u[:], mybir.AluOpType.mult, alu_b
                    )
                    # final comb: 4 diagonal matmuls, PSUM-accumulated
                    f = fpool.tile([128, 512], F32, tag="f")
                    mm(f[:], dg_t[0][:], a2[:, 0:512], True, False)
                    mm(f[:], dg_t[1][:], b2[:, 0:512], False, False)
                    mm(f[:], dg_t[2][:], a2[:, 1:513], False, False)
                    mm(f[:], dg_t[3][:], b2[:, 1:513], False, True)
                    # evict PSUM -> SBUF f32 (ACT)
                    nc.scalar.copy(og[:, j, :], f[:])

                nc.sync.dma_start(o_view[:, rp0 : rp0 + nrp, :], og[:, 0:nrp, :])

    return nc, go


def derive_consts(conv_w, bias, up_filter, down_filter):
    f = np.asarray(up_filter, dtype=np.float64).reshape(-1)
    d = np.asarray(down_filter, dtype=np.float64).reshape(-1)
    fk = (f * 4.0)[::-1]
    fd = d[::-1]
    assert abs(fk[1] - fk[3]) < 1e-6 * max(1.0, abs(fk[1])), "up filter not symmetric"
    assert abs(fk[0] - fk[4]) < 1e-6 * max(1.0, abs(fk[0])), "up filter not symmetric"
    fk0, fk1, fk2 = float(fk[0]), float(fk[1]), float(fk[2])
    assert fk0 != 0.0
    ratio = fk2 / fk0

    # partition index q = 2*ci + g (g = h-half); output partition 2*co + g
    cw = np.asarray(conv_w, dtype=np.float32)  # [co, ci, 1, K]
    wb = np.zeros((K, 128, 128), dtype=np.float16)
    for k in range(K):
        wk = cw[:, :, 0, k].T.astype(np.float16)  # [ci, co]
        wb[k, 0::2, 0::2] = wk
        wb[k, 1::2, 1::2] = wk

    brow = np.zeros((1, 128), dtype=np.float16)
    brow[0, :] = np.repeat(np.asarray(bias, dtype=np.float16), 2)

    pat = np.zeros((1, 515), dtype=np.float16)
    pat[0, 1:513] = 1.0  # cols 1..512 <-> m 0..511 valid

    eye = np.eye(128, dtype=np.float32)
    dg = np.stack(
        [
            np.float32(fd[0] * fk1) * eye,
            np.float32(fd[1] * fk0) * eye,
            np.float32(fd[2] * fk1) * eye,
            np.float32(fd[3] * fk0) * eye,
        ]
    ).astype(np.float16)

    return {
        "wb": wb,
        "brow": brow,
        "pat": pat,
        "dg": dg,
        "ratio": ratio,
        "alu_a": _alu(fk1),
        "alu_b": _alu(fk0),
    }


_CACHE = {}


def _get_compiled(consts_key, ratio, alu_a, alu_b):
    if consts_key in _CACHE:
        return _CACHE[consts_key]
    nc, go = build_program()
    go(ratio, alu_a, alu_b)
    nc.compile()
    _CACHE[consts_key] = nc
    return nc


def run(x, conv_w, bias, up_filter, down_filter, trace=False, **trace_kw):
    x = np.asarray(x, dtype=np.float32)
    c = derive_consts(conv_w, bias, up_filter, down_filter)

    key = (float(c["ratio"]), c["alu_a"].value, c["alu_b"].value)
    nc = _get_compiled(key, c["ratio"], c["alu_a"], c["alu_b"])

    in_maps = []
    for i in range(N_CORES):
        in_maps.append(
            {
                "x": np.ascontiguousarray(x[i]),
                "wb": c["wb"],
                "brow": c["brow"],
                "pat": c["pat"],
                "dg": c["dg"],
            }
        )
    res = run_bass_kernel_spmd(
        nc, in_maps, list(range(N_CORES)), trace=trace, **trace_kw
    )
    out = np.stack([res.results[i]["out"] for i in range(N_CORES)], axis=0)
    return out.astype(np.float32), res


def kernel(x, conv_w, bias, up_filter, down_filter):
    out, _ = run(x, conv_w, bias, up_filter, down_filter)
    return out
